# revision 1
# baseline (speedup 1.0000x reference)
"""Fused per-pixel kernel for nn_KernelFusion_19026705121450 on 8 trn2 cores.

Math: the reference computes, per pixel q = z[b,:,h,w] (3 channels):
    z_map = Wz q + bz                      (64-dim)
    t_b   = Wt text_b + bt                 (64-dim, per batch)
    dist  = ||z_map - t_b||^2
    k_lin = z_map . t_b
    k     = (w0 exp(-g*dist) + w1 k_lin + w2 (a k_lin + c)^2) / (sum w + 1e-8)
    out   = (1 + sigmoid(k)) * z_map -> 1x1 conv Wo + bo

Every 64-dim reduction is a quadratic/linear form in the 3-dim q, so on
host (fp64) we collapse:
    dist  = ||L^T q + r_b||^2 + rho_b      (L = chol(Wz^T Wz), 3x3)
    k_lin = u_b . q + s_b
    out_o = (1+sigmoid(k)) * (M q + m)_o + bo_o,  M = Wo Wz (3x3)

Device kernel is ~34 elementwise ops per tile; no matmuls, no HID dim.
Sharding: pure data parallel, 2 batches/core. On-chip layout [128, 1024]:
partition p -> (batch p//64, row p%64), free f -> pixel (p%64)*1024 + f.
Per-batch constants ride as [128,1] per-partition AP operands, so the
compiled program is input-independent (cached across calls).
"""

import sys

if "/opt/trn_rl_repo" not in sys.path:
    sys.path.insert(0, "/opt/trn_rl_repo")

import numpy as np

import concourse.bass as bass
import concourse.bacc as bacc
import concourse.mybir as mybir
from concourse.tile import TileContext
from concourse import bass_utils

F32 = mybir.dt.float32
AF = mybir.ActivationFunctionType
OP = mybir.AluOpType

NCORES = 8
BPC = 2          # batches per core
ROWS = 64        # partition rows per batch
P = 128          # partitions
FREE = 1024      # free dim (ROWS * FREE = H*W)
NCONST = 33

# const column indices
C_R0, C_R1, C_R2, C_BETA0, C_NEGG = 0, 1, 2, 3, 4
C_U0, C_U1, C_U2, C_S = 5, 6, 7, 8
C_L00, C_L10, C_L20, C_L11, C_L21, C_L22 = 9, 10, 11, 12, 13, 14
C_APOLY, C_BPOLY, C_W1P = 15, 16, 17
C_M = 18          # 18..26 row-major M[o][c]
C_MV = 27         # 27..29 m
C_BO = 30         # 30..32 out bias

_NC_CACHE: dict = {}


def _build_nc(sw0_pos: bool, sw2_pos: bool, nchunk: int = 2,
              use_gpsimd: bool = True, cfg: dict | None = None):
    # cfg knobs: d_eng, to_engs (3-tuple), oo_engs (3-tuple), g1_eng,
    # start_kla, start_e1  ('act'|'dve'|'pool')
    cfg = dict(cfg or {})
    d_eng = cfg.get("d_eng", "pool" if use_gpsimd else "dve")
    to_engs = cfg.get("to_engs", ("pool" if use_gpsimd else "dve",) * 3)
    oo_engs = cfg.get("oo_engs", ("act",) * 3)
    g1_eng = cfg.get("g1_eng", "act")
    start_kla = cfg.get("start_kla", "act")
    start_e1 = cfg.get("start_e1", "act")
    start_ya = cfg.get("start_ya", ("act",) * 3)
    inplace = cfg.get("inplace", False)
    wbufs_cfg = cfg.get("bufs", None)
    out_dma = cfg.get("out_dma", "sync")
    nc = bacc.Bacc("TRN2", target_bir_lowering=False)
    # packed input: [consts (NCONST) | chunk0: z0|z1|z2 | chunk1: z0|z1|z2]
    # one DMA per chunk -> one wait semaphore per chunk (walrus rejects
    # instructions with too many sync waits).
    cw = FREE // nchunk
    zc_cols = NCONST + 3 * FREE
    zc = nc.dram_tensor("zc", [P, zc_cols], F32, kind="ExternalInput")
    out = nc.dram_tensor("out_shard", [3, P, FREE], F32, kind="ExternalOutput")

    op_k2 = OP.add if sw2_pos else OP.subtract
    op_k1 = OP.add if sw0_pos else OP.subtract
    cf = FREE // nchunk

    def E(which):
        return {"act": nc.scalar, "dve": nc.vector, "pool": nc.gpsimd}[which]

    with TileContext(nc) as tc:
        with tc.tile_pool(name="cpool", bufs=1) as cpool, \
             tc.tile_pool(name="work", bufs=1) as pool:
            zt = cpool.tile([P, zc_cols], F32, name="zt")
            # chunk 0 DMA carries the consts columns too
            nc.sync.dma_start(out=zt[:, 0:NCONST + 3 * cw],
                              in_=zc[:, 0:NCONST + 3 * cw])
            for ci in range(1, nchunk):
                a = NCONST + 3 * cw * ci
                nc.sync.dma_start(out=zt[:, a:a + 3 * cw],
                                  in_=zc[:, a:a + 3 * cw])

            def col(j):
                return zt[:, j:j + 1]

            for ci in range(nchunk):
                fs = ci * cf
                sl = (slice(None), slice(fs, fs + cf))
                base = NCONST + 3 * cw * ci
                z0 = zt[:, base:base + cw]
                z1 = zt[:, base + cw:base + 2 * cw]
                z2 = zt[:, base + 2 * cw:base + 3 * cw]

                def t(tag):
                    return pool.tile([P, cf], F32, tag=f"{tag}_{ci}",
                                     name=f"{tag}_{ci}")

                # dist path: e = L^T q + r, dist = sum e_i^2 + rho
                e0 = t("e0")
                nc.scalar.activation(e0[:, :], z0[:, :], AF.Identity,
                                     bias=col(C_R0), scale=col(C_L00))
                e0b = t("e0b")
                nc.vector.scalar_tensor_tensor(e0b[:, :], z1[:, :], col(C_L10),
                                               e0[:, :], OP.mult, OP.add)
                e0c = e0b if inplace else t("e0c")
                nc.vector.scalar_tensor_tensor(e0c[:, :], z2[:, :], col(C_L20),
                                               e0b[:, :], OP.mult, OP.add)
                sq0 = t("sq0")
                nc.scalar.activation(sq0[:, :], e0c[:, :], AF.Square)

                e1 = t("e1")
                if start_e1 == "act":
                    nc.scalar.activation(e1[:, :], z1[:, :], AF.Identity,
                                         bias=col(C_R1), scale=col(C_L11))
                else:
                    E(start_e1).tensor_scalar(e1[:, :], z1[:, :], col(C_L11),
                                              col(C_R1), OP.mult, OP.add)
                e1b = t("e1b")
                nc.vector.scalar_tensor_tensor(e1b[:, :], z2[:, :], col(C_L21),
                                               e1[:, :], OP.mult, OP.add)
                sq1 = t("sq1")
                nc.scalar.activation(sq1[:, :], e1b[:, :], AF.Square)

                sq2 = t("sq2")
                nc.scalar.activation(sq2[:, :], z2[:, :], AF.Square,
                                     bias=col(C_R2), scale=col(C_L22))

                d1 = t("d1")
                d2 = t("d2")
                E(d_eng).tensor_add(out=d1[:, :], in0=sq0[:, :], in1=sq1[:, :])
                E(d_eng).tensor_add(out=d2[:, :], in0=d1[:, :], in1=sq2[:, :])

                # krbf = |w0'| * exp(-g*dist) via bias fold
                krbf = t("krbf")
                nc.scalar.activation(krbf[:, :], d2[:, :], AF.Exp,
                                     bias=col(C_BETA0), scale=col(C_NEGG))

                # k_lin = u . q + s
                kla = t("kla")
                if start_kla == "act":
                    nc.scalar.activation(kla[:, :], z0[:, :], AF.Identity,
                                         bias=col(C_S), scale=col(C_U0))
                else:
                    E(start_kla).tensor_scalar(kla[:, :], z0[:, :], col(C_U0),
                                               col(C_S), OP.mult, OP.add)
                klb = t("klb")
                nc.vector.scalar_tensor_tensor(klb[:, :], z1[:, :], col(C_U1),
                                               kla[:, :], OP.mult, OP.add)
                kl = klb if inplace else t("kl")
                nc.vector.scalar_tensor_tensor(kl[:, :], z2[:, :], col(C_U2),
                                               klb[:, :], OP.mult, OP.add)

                # p2 = |w2'| (a k_lin + c)^2
                p2 = t("p2")
                nc.scalar.activation(p2[:, :], kl[:, :], AF.Square,
                                     bias=col(C_BPOLY), scale=col(C_APOLY))

                kb = t("kb")
                nc.vector.scalar_tensor_tensor(kb[:, :], kl[:, :], col(C_W1P),
                                               p2[:, :], OP.mult, op_k2)
                kt = t("kt")
                if op_k1 == OP.add:
                    nc.vector.tensor_add(out=kt[:, :], in0=kb[:, :], in1=krbf[:, :])
                else:
                    nc.vector.tensor_sub(out=kt[:, :], in0=kb[:, :], in1=krbf[:, :])

                sig = t("sig")
                nc.scalar.activation(sig[:, :], kt[:, :], AF.Sigmoid)
                g1 = t("g1")
                if g1_eng == "act":
                    nc.scalar.activation(g1[:, :], sig[:, :], AF.Identity,
                                         bias=1.0)
                else:
                    E(g1_eng).tensor_scalar_add(g1[:, :], sig[:, :], 1.0)

                # y_o = M q + m ; out_o = g1*y_o + bo
                for o in range(3):
                    ya = t(f"ya{o}")
                    if start_ya[o] == "act":
                        nc.scalar.activation(ya[:, :], z0[:, :], AF.Identity,
                                             bias=col(C_MV + o),
                                             scale=col(C_M + 3 * o))
                    else:
                        E(start_ya[o]).tensor_scalar(ya[:, :], z0[:, :],
                                                     col(C_M + 3 * o),
                                                     col(C_MV + o),
                                                     OP.mult, OP.add)
                    yb = t(f"yb{o}")
                    nc.vector.scalar_tensor_tensor(yb[:, :], z1[:, :],
                                                   col(C_M + 3 * o + 1),
                                                   ya[:, :], OP.mult, OP.add)
                    yc = yb if inplace else t(f"yc{o}")
                    nc.vector.scalar_tensor_tensor(yc[:, :], z2[:, :],
                                                   col(C_M + 3 * o + 2),
                                                   yb[:, :], OP.mult, OP.add)
                    to = t(f"to{o}")
                    E(to_engs[o]).tensor_mul(out=to[:, :], in0=yc[:, :],
                                             in1=g1[:, :])
                    oo = to if inplace else t(f"oo{o}")
                    if oo_engs[o] == "act":
                        nc.scalar.activation(oo[:, :], to[:, :], AF.Identity,
                                             bias=col(C_BO + o))
                    else:
                        E(oo_engs[o]).tensor_scalar_add(oo[:, :], to[:, :],
                                                        col(C_BO + o))
                    dma_eng = nc.scalar if out_dma == "scalar" else nc.sync
                    dma_eng.dma_start(out=out[o][sl], in_=oo[:, :])
    nc.compile()
    return nc


def _get_nc(sw0_pos, sw2_pos, nchunk=2, use_gpsimd=True, cfg=None):
    key = (sw0_pos, sw2_pos, nchunk, use_gpsimd,
           tuple(sorted((cfg or {}).items())))
    if key not in _NC_CACHE:
        _NC_CACHE[key] = _build_nc(sw0_pos, sw2_pos, nchunk, use_gpsimd, cfg)
    return _NC_CACHE[key]


def _host_prep(inputs, nchunk=2):
    d = {k: np.asarray(v, dtype=np.float64) for k, v in inputs.items()}
    z = np.ascontiguousarray(np.asarray(inputs["z"], dtype=np.float32))
    B, C, H, W = z.shape
    HW = H * W
    Wz, bz = d["z_proj_w"], d["z_proj_b"]
    Wt, bt = d["text_proj_w"], d["text_proj_b"]
    Wo, bo = d["out_w"], d["out_b"]
    gamma = np.exp(d["log_gamma"])
    alpha, c, w = d["alpha"], d["c"], d["w"]
    sumw = w.sum() + 1e-8
    w0p, w1p, w2p = w[0] / sumw, w[1] / sumw, w[2] / sumw

    t = d["text_vec"] @ Wt.T + bt                       # [B, HID]
    G = Wz.T @ Wz                                       # [3,3]
    L = np.linalg.cholesky(G)                           # may raise -> caller
    delta = bz[None, :] - t                             # [B, HID]
    v = delta @ Wz                                      # [B, 3]
    cdist = (delta ** 2).sum(1)                         # [B]
    r = np.linalg.solve(L, v.T).T                       # [B, 3], L r = v
    rho = cdist - (r ** 2).sum(1)
    u = t @ Wz                                          # [B, 3]
    s = t @ bz                                          # [B]
    if w0p == 0.0:
        beta0 = np.full(B, -1e30)
    else:
        beta0 = -gamma * rho + np.log(abs(w0p))
    aPoly = alpha * np.sqrt(abs(w2p))
    bPoly = c * np.sqrt(abs(w2p))
    M = Wo @ Wz                                         # [3,3]
    m = Wo @ bz                                         # [3]

    cb = np.zeros((B, NCONST), dtype=np.float64)
    cb[:, C_R0], cb[:, C_R1], cb[:, C_R2] = r[:, 0], r[:, 1], r[:, 2]
    cb[:, C_BETA0] = beta0
    cb[:, C_NEGG] = -gamma
    cb[:, C_U0], cb[:, C_U1], cb[:, C_U2] = u[:, 0], u[:, 1], u[:, 2]
    cb[:, C_S] = s
    cb[:, C_L00], cb[:, C_L10], cb[:, C_L20] = L[0, 0], L[1, 0], L[2, 0]
    cb[:, C_L11], cb[:, C_L21], cb[:, C_L22] = L[1, 1], L[2, 1], L[2, 2]
    cb[:, C_APOLY], cb[:, C_BPOLY], cb[:, C_W1P] = aPoly, bPoly, w1p
    for o in range(3):
        cb[:, C_M + 3 * o: C_M + 3 * o + 3] = M[o]
        cb[:, C_MV + o] = m[o]
        cb[:, C_BO + o] = bo[o]
    cb = cb.astype(np.float32)

    cw = FREE // nchunk
    in_maps = []
    for core in range(NCORES):
        zs = np.empty((3, P, FREE), dtype=np.float32)
        cs = np.empty((P, NCONST), dtype=np.float32)
        for j in range(BPC):
            b = core * BPC + j
            zs[:, j * ROWS:(j + 1) * ROWS, :] = z[b].reshape(3, ROWS, FREE)
            cs[j * ROWS:(j + 1) * ROWS, :] = cb[b]
        packed = np.empty((P, NCONST + 3 * FREE), dtype=np.float32)
        packed[:, :NCONST] = cs
        for ci in range(nchunk):
            base = NCONST + 3 * cw * ci
            for c in range(3):
                packed[:, base + c * cw:base + (c + 1) * cw] = \
                    zs[c, :, ci * cw:(ci + 1) * cw]
        in_maps.append({"zc": packed})
    return in_maps, (w0p >= 0.0, w2p >= 0.0), (B, C, H, W)


def _numpy_fallback(inputs):
    d = {k: np.asarray(v, dtype=np.float64) for k, v in inputs.items()}
    z, Wz, bz = d["z"], d["z_proj_w"], d["z_proj_b"]
    t = d["text_vec"] @ d["text_proj_w"].T + d["text_proj_b"]
    zm = np.einsum("bchw,oc->bohw", z, Wz) + bz[None, :, None, None]
    gamma = np.exp(d["log_gamma"])
    diff = zm - t[:, :, None, None]
    dist = (diff * diff).sum(1)
    klin = np.einsum("bchw,bc->bhw", zm, t)
    krbf = np.exp(-gamma * dist)
    kpoly = (d["alpha"] * klin + d["c"]) ** 2
    w = d["w"]
    k = (w[0] * krbf + w[1] * klin + w[2] * kpoly) / (w.sum() + 1e-8)
    zf = zm * (1.0 + 1.0 / (1.0 + np.exp(-k[:, None])))
    out = np.einsum("bchw,oc->bohw", zf, d["out_w"]) + d["out_b"][None, :, None, None]
    return out.astype(np.float32)


BEST_CFG: dict = {"d_eng": "dve", "to_engs": ("dve", "dve", "dve")}
BEST_NCHUNK = 2
BEST_GPSIMD = False


def run(inputs, trace=False, nchunk=None, use_gpsimd=None, cfg=None):
    if nchunk is None:
        nchunk = BEST_NCHUNK
    if use_gpsimd is None:
        use_gpsimd = BEST_GPSIMD
    if cfg is None:
        cfg = BEST_CFG
    try:
        in_maps, (sw0, sw2), (B, C, H, W) = _host_prep(inputs, nchunk)
    except np.linalg.LinAlgError:
        return _numpy_fallback(inputs), None
    nc = _get_nc(sw0, sw2, nchunk, use_gpsimd, cfg)
    res = bass_utils.run_bass_kernel_spmd(
        nc, in_maps, core_ids=list(range(NCORES)), trace=trace)
    out = np.empty((B, C, H, W), dtype=np.float32)
    for core in range(NCORES):
        o = res.results[core]["out_shard"]          # [3, P, FREE]
        for j in range(BPC):
            b = core * BPC + j
            out[b] = o[:, j * ROWS:(j + 1) * ROWS, :].reshape(C, H, W)
    return out, res


def kernel(**inputs):
    out, _ = run(inputs, trace=False)
    return out



# revision 13
# speedup vs baseline: 1.5424x; 1.5424x over previous
"""Fused per-pixel kernel for nn_KernelFusion_19026705121450 on 8 trn2 cores.

Math: per pixel q = z[b,:,h,w] (3 ch), per batch t = Wt text + bt:
    z_map = Wz q + bz; dist = ||z_map - t||^2; kl = z_map . t
    k = (w0 e^{-g dist} + w1 kl + w2 (a kl + c)^2) / (sum w + 1e-8)
    out = Wo (z_map (1 + sigmoid(k))) + bo

All 64-dim reductions collapse (host, fp64) to 3-dim forms:
    dist = ||L^T q + r||^2 + rho   (L = chol(Wz^T Wz))
    kl   = u . q + s
    out_o = (M_o . q + m_o) g + bo_o,  M = Wo Wz, g = 1.5 + 0.5 tanh(k/2)

Device: one 1024-col pass over [128, 1024] fp16 tiles (partition =
batch*64 + rowblock, free = pixel). Forms are pivot-normalized on their
lead channel so biases ride tensor_scalar const slots; pivot scales
refold into ACT Square scales / per-o g1 consts. tanh (same ACT table
as exp/square) replaces sigmoid to avoid a table reload; a warmup ACT
op preloads the table before DMAs land. MACs decompose per cfg across
DVE (ts+tt / stt / ln_bwd custom op) and Pool (ts half).
"""

import sys

if "/opt/trn_rl_repo" not in sys.path:
    sys.path.insert(0, "/opt/trn_rl_repo")

import numpy as np

import concourse.bass as bass
import concourse.bacc as bacc
import concourse.mybir as mybir
from concourse.tile import TileContext
from concourse import bass_utils

F32 = mybir.dt.float32
F16 = mybir.dt.float16
AF = mybir.ActivationFunctionType
OP = mybir.AluOpType

NCORES = 8
BPC = 2          # batches per core
ROWS = 64        # partition rows per batch
P = 128
FREE = 1024

# const column indices (fp32 tensor)
# form f: z_lead + a1*z_a + a2*z_b + bias  (negated copies for ln mode)
C_A1E0, C_BE0, C_A2E0, C_SQ0S = 0, 1, 2, 3
C_A1E1, C_BE1, C_SQ1S = 4, 5, 6
C_SQ2S, C_SQ2B = 7, 8
C_NEGG, C_BETA0 = 9, 10
C_A1KL, C_BKL, C_A2KL = 11, 12, 13
C_P2S, C_P2B, C_W1U0 = 14, 15, 16
C_A1Y, C_BY, C_A2Y = 17, 20, 23       # +o
C_G1S, C_G1B = 26, 29                 # +o
C_BO = 32                             # +o
C_NA1E0, C_NBE0 = 35, 36              # negated (for ln_bwd mode)
C_NA1E1, C_NBE1 = 37, 38
C_NA1KL, C_NBKL = 39, 40
C_NA1Y, C_NBY = 41, 44                # +o
NCONST = 47

_NC_CACHE: dict = {}

BEST_CFG: dict = {"in_eng": ("act", "sync", "sync", "sync"),
                  "zorder": "z2split"}


def _build_nc(sw0_pos: bool, sw2_pos: bool, bo_zero: bool, cfg: dict | None):
    """Emission order is hand-scheduled for the in-order engines.

    Step names (used by the `plan` cfg: list of (step, engine) pairs, where
    engine is 'dve'|'pool'|'act' for compute placement where it matters):
      mul ops ("<form>m1" = ts of z_aux w/ bias, "<form>m2" = ts of z2),
      adds ("<form>a1" lead+m1, "<form>a2" +m2), squares/exp/tanh on ACT
      fixed, d1/d2/t2 adds, tpoly stt, g1_o, v_o.
    """
    cfg = dict(cfg or {})
    warm = cfg.get("warm", True)
    # consts, z1, z0, z2 DMA queues
    in_eng = cfg.get("in_eng", ("act", "sync", "sync", "pool"))
    out_eng = cfg.get("out_eng", ("sync", "pool", "act"))
    # placement of the movable mul/aux ops
    pool_ops = set(cfg.get("pool_ops",
                           ("e0m2", "klm2", "y1m1", "y2m1", "y1m2",
                            "y2m2")))
    act_ops = set(cfg.get("act_ops", ("e1m1", "y0m2")))
    dve_order = cfg.get("dve_order", None)
    g1_act = set(cfg.get("g1_act", (1, 2)))     # g1 indices on ACT

    nc = bacc.Bacc("TRN2", target_bir_lowering=False)
    cons = nc.dram_tensor("consts", [P, NCONST], F32, kind="ExternalInput")
    z01 = nc.dram_tensor("z01", [P, 2 * FREE], F16, kind="ExternalInput")
    z2d = nc.dram_tensor("z2", [P, FREE], F16, kind="ExternalInput")
    outs = [nc.dram_tensor(f"o{o}", [P, FREE], F16, kind="ExternalOutput")
            for o in range(3)]

    op_w2 = OP.add if sw2_pos else OP.subtract

    def dmaeng(which):
        return {"sync": nc.sync, "pool": nc.gpsimd, "act": nc.scalar,
                "dve": nc.vector}[which]

    with TileContext(nc) as tc:
        with tc.tile_pool(name="cpool", bufs=1) as cpool, \
             tc.tile_pool(name="work", bufs=1) as pool:
            ct = cpool.tile([P, NCONST], F32, name="ct")
            zt = cpool.tile([P, 2 * FREE], F16, name="zt")
            z2t = cpool.tile([P, FREE], F16, name="z2t")
            wt = cpool.tile([P, 1], F32, name="wt")

            if warm:
                nc.vector.memset(wt[:, :], 0.0)
                nc.scalar.activation(wt[:, :], wt[:, :], AF.Square)

            # packed z01 = [z1 | z0]
            zorder = cfg.get("zorder", "z2first")
            dmaeng(in_eng[0]).dma_start(out=ct[:, :], in_=cons[:, :])
            if zorder == "z2first":
                dmaeng(in_eng[3]).dma_start(out=z2t[:, :], in_=z2d[:, :])
                dmaeng(in_eng[1]).dma_start(out=zt[:, :], in_=z01[:, :])
            elif zorder == "z01first":
                dmaeng(in_eng[1]).dma_start(out=zt[:, :], in_=z01[:, :])
                dmaeng(in_eng[3]).dma_start(out=z2t[:, :], in_=z2d[:, :])
            elif zorder == "split":
                dmaeng(in_eng[1]).dma_start(out=zt[:, 0:FREE],
                                            in_=z01[:, 0:FREE])
                dmaeng(in_eng[3]).dma_start(out=z2t[:, :], in_=z2d[:, :])
                dmaeng(in_eng[2]).dma_start(out=zt[:, FREE:2 * FREE],
                                            in_=z01[:, FREE:2 * FREE])
            else:  # z2split: z2, z1, z0 (all split)
                dmaeng(in_eng[3]).dma_start(out=z2t[:, :], in_=z2d[:, :])
                dmaeng(in_eng[1]).dma_start(out=zt[:, 0:FREE],
                                            in_=z01[:, 0:FREE])
                dmaeng(in_eng[2]).dma_start(out=zt[:, FREE:2 * FREE],
                                            in_=z01[:, FREE:2 * FREE])
            z1 = zt[:, 0:FREE]
            z0 = zt[:, FREE:2 * FREE]
            z2 = z2t[:, :]

            def col(j):
                return ct[:, j:j + 1]

            tiles = {}

            def t(tag, w=FREE):
                if tag not in tiles:
                    tiles[tag] = pool.tile([P, w], F16, tag=tag, name=tag)
                return tiles[tag]

            def E(tag):
                return nc.gpsimd if tag in pool_ops else nc.vector

            def ts_op(tag, dst, src, scol, bcol):
                # dst = scol*src + bcol on pool/act/dve per placement
                if tag in act_ops:
                    nc.scalar.activation(dst, src, AF.Identity,
                                         bias=bcol if not isinstance(bcol, float)
                                         else bcol, scale=scol)
                else:
                    E(tag).tensor_scalar(dst, src, scol, bcol,
                                         OP.mult, OP.add)

            # ---------- op emitters (callable in any order) ----------
            # forms: f in {e0, e1, kl, y0, y1, y2}
            # e1 is 2-term: lead z1, aux z2. others: lead z0, aux z1 + z2.
            FORM = {
                "e0": (None, C_A1E0, C_BE0, C_A2E0, C_NA1E0, C_NBE0),
                "kl": (None, C_A1KL, C_BKL, C_A2KL, C_NA1KL, C_NBKL),
                "y0": (None, C_A1Y + 0, C_BY + 0, C_A2Y + 0, C_NA1Y, C_NBY),
                "y1": (None, C_A1Y + 1, C_BY + 1, C_A2Y + 1, C_NA1Y + 1,
                       C_NBY + 1),
                "y2": (None, C_A1Y + 2, C_BY + 2, C_A2Y + 2, C_NA1Y + 2,
                       C_NBY + 2),
            }

            def m1(f):      # tmp = a1*z1 + b   (aux mul with bias)
                _, a1, b, _, _, _ = FORM[f]
                ts_op(f + "m1", t(f + "_m1")[:, :], z1, col(a1), col(b))

            def a1(f):      # acc = z0 + tmp
                nc.vector.tensor_add(out=t(f + "_a")[:, :], in0=z0,
                                     in1=t(f + "_m1")[:, :])

            def ln1(f):     # acc = z0 + a1*z1 + b via custom op (1 DVE op)
                _, _, _, _, na1, nb = FORM[f]
                nc.vector.ln_bwd_dx(t(f + "_a")[:, :], z0, z1, col(na1),
                                    col(nb))

            def m2(f):      # tmp2 = a2*z2
                _, _, _, a2, _, _ = FORM[f]
                ts_op(f + "m2", t(f + "_m2")[:, :], z2, col(a2), 0.0)

            def a2(f):      # out = acc + tmp2
                E(f + "a2").tensor_add(out=t(f)[:, :], in0=t(f + "_a")[:, :],
                                       in1=t(f + "_m2")[:, :])

            def s2(f):      # out = acc + a2*z2 via stt (skip m2)
                _, _, _, a2c, _, _ = FORM[f]
                nc.vector.scalar_tensor_tensor(t(f)[:, :], z2, col(a2c),
                                               t(f + "_a")[:, :], OP.mult,
                                               OP.add)

            def e1m(_=None):   # e1 aux: tmp = a1*z2 + b
                ts_op("e1m1", t("e1_m1")[:, :], z2, col(C_A1E1),
                      col(C_BE1))

            def e1a(_=None):   # e1 = z1 + tmp
                nc.vector.tensor_add(out=t("e1")[:, :], in0=z1,
                                     in1=t("e1_m1")[:, :])

            def e1ln(_=None):
                nc.vector.ln_bwd_dx(t("e1")[:, :], z1, z2, col(C_NA1E1),
                                    col(C_NBE1))

            def sq(i):      # ACT squares: 0 <- e0, 1 <- e1, 2 <- z2
                src = {0: t("e0"), 1: t("e1")}.get(i)
                scol = {0: C_SQ0S, 1: C_SQ1S, 2: C_SQ2S}[i]
                dst = t(f"sq{i}")
                if i == 2:
                    nc.scalar.activation(dst[:, :], z2, AF.Square,
                                         bias=col(C_SQ2B), scale=col(scol))
                else:
                    nc.scalar.activation(dst[:, :], src[:, :], AF.Square,
                                         scale=col(scol))

            def d1(_=None):
                E("d1").tensor_add(out=t("d1")[:, :], in0=t("sq1")[:, :],
                                   in1=t("sq2")[:, :])

            def d2(_=None):
                E("d2").tensor_add(out=t("d2")[:, :], in0=t("d1")[:, :],
                                   in1=t("sq0")[:, :])

            def krbf(_=None):
                nc.scalar.activation(t("krbf")[:, :], t("d2")[:, :], AF.Exp,
                                     bias=col(C_BETA0), scale=col(C_NEGG))

            def p2(_=None):
                nc.scalar.activation(t("p2")[:, :], t("kl")[:, :], AF.Square,
                                     bias=col(C_P2B), scale=col(C_P2S))

            def tpoly(_=None):
                nc.vector.scalar_tensor_tensor(t("tpoly")[:, :],
                                               t("kl")[:, :], col(C_W1U0),
                                               t("p2")[:, :], OP.mult, op_w2)

            def tpm(_=None):
                ts_op("tpm", t("tp_m")[:, :], t("kl")[:, :], col(C_W1U0), 0.0)

            def tpa(_=None):
                if sw2_pos:
                    E("tpa").tensor_add(out=t("tpoly")[:, :],
                                        in0=t("tp_m")[:, :],
                                        in1=t("p2")[:, :])
                else:
                    E("tpa").tensor_sub(out=t("tpoly")[:, :],
                                        in0=t("tp_m")[:, :],
                                        in1=t("p2")[:, :])

            def t2(_=None):
                if sw0_pos:
                    E("t2").tensor_add(out=t("t2")[:, :],
                                       in0=t("tpoly")[:, :],
                                       in1=t("krbf")[:, :])
                else:
                    E("t2").tensor_sub(out=t("t2")[:, :],
                                       in0=t("tpoly")[:, :],
                                       in1=t("krbf")[:, :])

            def th(_=None):
                nc.scalar.activation(t("th")[:, :], t("t2")[:, :], AF.Tanh,
                                     scale=0.5)

            def g1(o):
                if o in g1_act:
                    nc.scalar.activation(t(f"g1{o}")[:, :], t("th")[:, :],
                                         AF.Identity, bias=col(C_G1B + o),
                                         scale=col(C_G1S + o))
                else:
                    E(f"g1{o}").tensor_scalar(t(f"g1{o}")[:, :],
                                              t("th")[:, :], col(C_G1S + o),
                                              col(C_G1B + o), OP.mult,
                                              OP.add)

            def v(o, nsplit=1):
                vt = t(f"v{o}")
                cw = FREE // nsplit
                for sdx in range(nsplit):
                    sl = (slice(None), slice(sdx * cw, (sdx + 1) * cw))
                    E(f"v{o}").tensor_mul(out=vt[sl], in0=t(f"y{o}")[sl],
                                          in1=t(f"g1{o}")[sl])
                    fin = vt
                    if not bo_zero:
                        fin = t(f"f{o}")
                        nc.vector.tensor_scalar(fin[sl], vt[sl], 1.0,
                                                col(C_BO + o), OP.mult,
                                                OP.add)
                    dmaeng(out_eng[o]).dma_start(out=outs[o][sl],
                                                 in_=fin[sl])

            # ---------- emission schedule ----------
            # Pool-assigned mul ops are emitted when their step comes up;
            # engine in-order sequencing follows emission order per engine.
            steps = {
                "e1m": e1m, "e1a": e1a, "e1ln": e1ln, "d1": d1, "d2": d2,
                "krbf": krbf, "p2": p2, "tpoly": tpoly, "tpm": tpm,
                "tpa": tpa, "t2": t2, "th": th,
            }
            for f in FORM:
                steps[f + "m1"] = (lambda ff: lambda _=None: m1(ff))(f)
                steps[f + "a1"] = (lambda ff: lambda _=None: a1(ff))(f)
                steps[f + "ln1"] = (lambda ff: lambda _=None: ln1(ff))(f)
                steps[f + "m2"] = (lambda ff: lambda _=None: m2(ff))(f)
                steps[f + "a2"] = (lambda ff: lambda _=None: a2(ff))(f)
                steps[f + "s2"] = (lambda ff: lambda _=None: s2(ff))(f)
            for o in range(3):
                steps[f"sq{o}"] = (lambda oo: lambda _=None: sq(oo))(o)
                steps[f"g1{o}"] = (lambda oo: lambda _=None: g1(oo))(o)
                steps[f"v{o}"] = (lambda oo: lambda _=None: v(oo))(o)
                steps[f"v{o}s"] = (lambda oo: lambda _=None: v(oo, 2))(o)

            if dve_order is None:
                dve_order = DEFAULT_ORDER
            for s in dve_order:
                steps[s]()
    nc.compile()
    return nc


# Default schedule: e1 first (z2 arrives early on pool queue), kl path
# early (feeds p2 before krbf), e0 path, squares interleave on ACT, y
# mac work fills DVE while ACT runs, tail g/v lanes. Pool-assigned ops
# appear in the order too (per-engine in-order follows emission order).
DEFAULT_ORDER = (
    "klm1", "e0m1", "y0m1",           # dve ts of z1 (start asap)
    "sq2", "e1m", "y0m2",             # act: z2 square + offloaded affines
    "e0m2", "klm2", "y1m1", "y2m1", "y1m2", "y2m2",   # pool ts queue
    "kla1", "e1a", "e0a1",
    "sq1",
    "kla2", "e0a2",
    "sq0",
    "y0a1", "d1", "d2",
    "p2", "krbf",
    "y1a1", "tpoly", "t2",
    "th",
    "y0a2", "y1a2", "y2a1", "y2a2",
    "g10", "g11", "g12",
    "v0", "v1", "v2",
)


def _get_nc(sw0_pos, sw2_pos, bo_zero, cfg=None):
    def freeze(v):
        if isinstance(v, dict):
            return tuple(sorted(v.items()))
        return v
    key = (sw0_pos, sw2_pos, bo_zero,
           tuple(sorted((k, freeze(v)) for k, v in (cfg or {}).items())))
    if key not in _NC_CACHE:
        _NC_CACHE[key] = _build_nc(sw0_pos, sw2_pos, bo_zero, cfg)
    return _NC_CACHE[key]


def _host_prep(inputs):
    d = {k: np.asarray(v, dtype=np.float64) for k, v in inputs.items()}
    z = np.asarray(inputs["z"], dtype=np.float32)
    B, C, H, W = z.shape
    Wz, bz = d["z_proj_w"], d["z_proj_b"]
    Wt, bt = d["text_proj_w"], d["text_proj_b"]
    Wo, bo = d["out_w"], d["out_b"]
    gamma = np.exp(d["log_gamma"])
    alpha, c, w = d["alpha"], d["c"], d["w"]
    sumw = w.sum() + 1e-8
    w0p, w1p, w2p = w[0] / sumw, w[1] / sumw, w[2] / sumw

    t = d["text_vec"] @ Wt.T + bt                      # [B, HID]
    G = Wz.T @ Wz
    L = np.linalg.cholesky(G)                          # may raise
    delta = bz[None, :] - t
    v = delta @ Wz                                     # [B, 3]
    cdist = (delta ** 2).sum(1)
    r = np.linalg.solve(L, v.T).T                      # [B, 3]
    rho = cdist - (r ** 2).sum(1)
    u = t @ Wz                                         # [B, 3]
    s = t @ bz                                         # [B]
    M = Wo @ Wz                                        # [3, 3]
    m = Wo @ bz                                        # [3]

    u0 = u[:, 0]
    piv = [L[0, 0], L[1, 1]] + [M[o, 0] for o in range(3)]
    if min(abs(np.asarray(piv))) < 1e-7 or np.any(np.abs(u0) < 1e-7):
        raise np.linalg.LinAlgError("degenerate pivot")

    if w0p == 0.0:
        beta0 = np.full(B, -1e30)
    else:
        beta0 = -gamma * rho + np.log(abs(w0p))
    sw2 = np.sqrt(abs(w2p))

    cb = np.zeros((B, NCONST), dtype=np.float64)
    cb[:, C_A1E0] = L[1, 0] / L[0, 0]
    cb[:, C_BE0] = r[:, 0] / L[0, 0]
    cb[:, C_A2E0] = L[2, 0] / L[0, 0]
    cb[:, C_SQ0S] = L[0, 0]
    cb[:, C_A1E1] = L[2, 1] / L[1, 1]
    cb[:, C_BE1] = r[:, 1] / L[1, 1]
    cb[:, C_SQ1S] = L[1, 1]
    cb[:, C_SQ2S] = L[2, 2]
    cb[:, C_SQ2B] = r[:, 2]
    cb[:, C_NEGG] = -gamma
    cb[:, C_BETA0] = beta0
    cb[:, C_A1KL] = u[:, 1] / u0
    cb[:, C_BKL] = s / u0
    cb[:, C_A2KL] = u[:, 2] / u0
    cb[:, C_P2S] = alpha * sw2 * u0
    cb[:, C_P2B] = c * sw2
    cb[:, C_W1U0] = w1p * u0
    for o in range(3):
        cb[:, C_A1Y + o] = M[o, 1] / M[o, 0]
        cb[:, C_BY + o] = m[o] / M[o, 0]
        cb[:, C_A2Y + o] = M[o, 2] / M[o, 0]
        cb[:, C_G1S + o] = 0.5 * M[o, 0]
        cb[:, C_G1B + o] = 1.5 * M[o, 0]
        cb[:, C_BO + o] = bo[o]
    # negated copies for ln_bwd (out = in0 - in1*s0 - s1)
    cb[:, C_NA1E0] = -cb[:, C_A1E0]
    cb[:, C_NBE0] = -cb[:, C_BE0]
    cb[:, C_NA1E1] = -cb[:, C_A1E1]
    cb[:, C_NBE1] = -cb[:, C_BE1]
    cb[:, C_NA1KL] = -cb[:, C_A1KL]
    cb[:, C_NBKL] = -cb[:, C_BKL]
    for o in range(3):
        cb[:, C_NA1Y + o] = -cb[:, C_A1Y + o]
        cb[:, C_NBY + o] = -cb[:, C_BY + o]
    cb = cb.astype(np.float32)

    z16 = z.astype(np.float16)
    in_maps = []
    for core in range(NCORES):
        cs = np.empty((P, NCONST), dtype=np.float32)
        z01a = np.empty((P, 2 * FREE), dtype=np.float16)
        z2a = np.empty((P, FREE), dtype=np.float16)
        for j in range(BPC):
            b = core * BPC + j
            pl = z16[b].reshape(3, ROWS, FREE)
            rs = slice(j * ROWS, (j + 1) * ROWS)
            z01a[rs, 0:FREE] = pl[1]
            z01a[rs, FREE:2 * FREE] = pl[0]
            z2a[rs, :] = pl[2]
            cs[rs, :] = cb[b]
        in_maps.append({"consts": cs, "z01": z01a, "z2": z2a})
    flags = (bool(w0p >= 0.0), bool(w2p >= 0.0),
             bool(np.all(bo == 0.0)))
    return in_maps, flags, (B, C, H, W)


def _numpy_fallback(inputs):
    d = {k: np.asarray(v, dtype=np.float64) for k, v in inputs.items()}
    z, Wz, bz = d["z"], d["z_proj_w"], d["z_proj_b"]
    t = d["text_vec"] @ d["text_proj_w"].T + d["text_proj_b"]
    zm = np.einsum("bchw,oc->bohw", z, Wz) + bz[None, :, None, None]
    gamma = np.exp(d["log_gamma"])
    diff = zm - t[:, :, None, None]
    dist = (diff * diff).sum(1)
    klin = np.einsum("bchw,bc->bhw", zm, t)
    krbf = np.exp(-gamma * dist)
    kpoly = (d["alpha"] * klin + d["c"]) ** 2
    w = d["w"]
    k = (w[0] * krbf + w[1] * klin + w[2] * kpoly) / (w.sum() + 1e-8)
    zf = zm * (1.0 + 1.0 / (1.0 + np.exp(-k[:, None])))
    out = np.einsum("bchw,oc->bohw", zf, d["out_w"]) + d["out_b"][None, :, None, None]
    return out.astype(np.float32)


def run(inputs, trace=False, cfg=None):
    if cfg is None:
        cfg = BEST_CFG
    try:
        in_maps, (sw0, sw2, boz), (B, C, H, W) = _host_prep(inputs)
    except np.linalg.LinAlgError:
        return _numpy_fallback(inputs), None
    nc = _get_nc(sw0, sw2, boz, cfg)
    res = bass_utils.run_bass_kernel_spmd(
        nc, in_maps, core_ids=list(range(NCORES)), trace=trace)
    out = np.empty((B, C, H, W), dtype=np.float32)
    for core in range(NCORES):
        r = res.results[core]
        for j in range(BPC):
            b = core * BPC + j
            rs = slice(j * ROWS, (j + 1) * ROWS)
            for o in range(3):
                out[b, o] = np.asarray(r[f"o{o}"][rs, :],
                                       dtype=np.float32).reshape(H, W)
    return out, res


def kernel(**inputs):
    out, _ = run(inputs, trace=False)
    return out


# revision 14
# speedup vs baseline: 1.5721x; 1.0192x over previous
"""Fused per-pixel kernel for nn_KernelFusion_19026705121450 on 8 trn2 cores.

Math: per pixel q = z[b,:,h,w] (3 ch), per batch t = Wt text + bt:
    z_map = Wz q + bz; dist = ||z_map - t||^2; kl = z_map . t
    k = (w0 e^{-g dist} + w1 kl + w2 (a kl + c)^2) / (sum w + 1e-8)
    out = Wo (z_map (1 + sigmoid(k))) + bo

All 64-dim reductions collapse (host, fp64) to 3-dim forms:
    dist = ||L^T q + r||^2 + rho   (L = chol(Wz^T Wz))
    kl   = u . q + s
    out_o = (M_o . q + m_o) g + bo_o,  M = Wo Wz, g = 1.5 + 0.5 tanh(k/2)

Device: one 1024-col pass over [128, 1024] fp16 tiles (partition =
batch*64 + rowblock, free = pixel). Forms are pivot-normalized on their
lead channel so biases ride tensor_scalar const slots; pivot scales
refold into ACT Square scales / per-o g1 consts. tanh (same ACT table
as exp/square) replaces sigmoid to avoid a table reload; a warmup ACT
op preloads the table before DMAs land. MACs decompose per cfg across
DVE (ts+tt / stt / ln_bwd custom op) and Pool (ts half).
"""

import sys

if "/opt/trn_rl_repo" not in sys.path:
    sys.path.insert(0, "/opt/trn_rl_repo")

import numpy as np

import concourse.bass as bass
import concourse.bacc as bacc
import concourse.mybir as mybir
from concourse.tile import TileContext
from concourse import bass_utils

F32 = mybir.dt.float32
F16 = mybir.dt.float16
AF = mybir.ActivationFunctionType
OP = mybir.AluOpType

NCORES = 8
BPC = 2          # batches per core
ROWS = 64        # partition rows per batch
P = 128
FREE = 1024

# const column indices (fp32 tensor)
# form f: z_lead + a1*z_a + a2*z_b + bias  (negated copies for ln mode)
C_A1E0, C_BE0, C_A2E0, C_SQ0S = 0, 1, 2, 3
C_A1E1, C_BE1, C_SQ1S = 4, 5, 6
C_SQ2S, C_SQ2B = 7, 8
C_NEGG, C_BETA0 = 9, 10
C_A1KL, C_BKL, C_A2KL = 11, 12, 13
C_P2S, C_P2B, C_W1U0 = 14, 15, 16
C_A1Y, C_BY, C_A2Y = 17, 20, 23       # +o
C_G1S, C_G1B = 26, 29                 # +o
C_BO = 32                             # +o
C_NA1E0, C_NBE0 = 35, 36              # negated (for ln_bwd mode)
C_NA1E1, C_NBE1 = 37, 38
C_NA1KL, C_NBKL = 39, 40
C_NA1Y, C_NBY = 41, 44                # +o
NCONST = 47

_NC_CACHE: dict = {}

BEST_CFG: dict = {"in_eng": ("act", "sync", "sync", "sync"),
                  "zorder": "z2split", "act_ops": (),
                  "out_eng": ("sync", "sync", "sync")}


def _build_nc(sw0_pos: bool, sw2_pos: bool, bo_zero: bool, cfg: dict | None):
    """Emission order is hand-scheduled for the in-order engines.

    Step names (used by the `plan` cfg: list of (step, engine) pairs, where
    engine is 'dve'|'pool'|'act' for compute placement where it matters):
      mul ops ("<form>m1" = ts of z_aux w/ bias, "<form>m2" = ts of z2),
      adds ("<form>a1" lead+m1, "<form>a2" +m2), squares/exp/tanh on ACT
      fixed, d1/d2/t2 adds, tpoly stt, g1_o, v_o.
    """
    cfg = dict(cfg or {})
    warm = cfg.get("warm", True)
    # consts, z1, z0, z2 DMA queues
    in_eng = cfg.get("in_eng", ("act", "sync", "sync", "pool"))
    out_eng = cfg.get("out_eng", ("sync", "pool", "act"))
    # placement of the movable mul/aux ops
    pool_ops = set(cfg.get("pool_ops",
                           ("e0m2", "klm2", "y1m1", "y2m1", "y1m2",
                            "y2m2")))
    act_ops = set(cfg.get("act_ops", ("e1m1", "y0m2")))
    dve_order = cfg.get("dve_order", None)
    g1_act = set(cfg.get("g1_act", (1, 2)))     # g1 indices on ACT

    nc = bacc.Bacc("TRN2", target_bir_lowering=False)
    cons = nc.dram_tensor("consts", [P, NCONST], F32, kind="ExternalInput")
    z01 = nc.dram_tensor("z01", [P, 2 * FREE], F16, kind="ExternalInput")
    z2d = nc.dram_tensor("z2", [P, FREE], F16, kind="ExternalInput")
    outs = [nc.dram_tensor(f"o{o}", [P, FREE], F16, kind="ExternalOutput")
            for o in range(3)]

    op_w2 = OP.add if sw2_pos else OP.subtract

    def dmaeng(which):
        return {"sync": nc.sync, "pool": nc.gpsimd, "act": nc.scalar,
                "dve": nc.vector}[which]

    with TileContext(nc) as tc:
        with tc.tile_pool(name="cpool", bufs=1) as cpool, \
             tc.tile_pool(name="work", bufs=1) as pool:
            ct = cpool.tile([P, NCONST], F32, name="ct")
            zt = cpool.tile([P, 2 * FREE], F16, name="zt")
            z2t = cpool.tile([P, FREE], F16, name="z2t")
            wt = cpool.tile([P, 1], F32, name="wt")

            if warm:
                nc.vector.memset(wt[:, :], 0.0)
                nc.scalar.activation(wt[:, :], wt[:, :], AF.Square)

            # packed z01 = [z1 | z0]
            zorder = cfg.get("zorder", "z2first")
            dmaeng(in_eng[0]).dma_start(out=ct[:, :], in_=cons[:, :])
            if zorder == "z2first":
                dmaeng(in_eng[3]).dma_start(out=z2t[:, :], in_=z2d[:, :])
                dmaeng(in_eng[1]).dma_start(out=zt[:, :], in_=z01[:, :])
            elif zorder == "z01first":
                dmaeng(in_eng[1]).dma_start(out=zt[:, :], in_=z01[:, :])
                dmaeng(in_eng[3]).dma_start(out=z2t[:, :], in_=z2d[:, :])
            elif zorder == "split":
                dmaeng(in_eng[1]).dma_start(out=zt[:, 0:FREE],
                                            in_=z01[:, 0:FREE])
                dmaeng(in_eng[3]).dma_start(out=z2t[:, :], in_=z2d[:, :])
                dmaeng(in_eng[2]).dma_start(out=zt[:, FREE:2 * FREE],
                                            in_=z01[:, FREE:2 * FREE])
            else:  # z2split: z2, z1, z0 (all split)
                dmaeng(in_eng[3]).dma_start(out=z2t[:, :], in_=z2d[:, :])
                dmaeng(in_eng[1]).dma_start(out=zt[:, 0:FREE],
                                            in_=z01[:, 0:FREE])
                dmaeng(in_eng[2]).dma_start(out=zt[:, FREE:2 * FREE],
                                            in_=z01[:, FREE:2 * FREE])
            z1 = zt[:, 0:FREE]
            z0 = zt[:, FREE:2 * FREE]
            z2 = z2t[:, :]

            def col(j):
                return ct[:, j:j + 1]

            tiles = {}

            def t(tag, w=FREE):
                if tag not in tiles:
                    tiles[tag] = pool.tile([P, w], F16, tag=tag, name=tag)
                return tiles[tag]

            def E(tag):
                return nc.gpsimd if tag in pool_ops else nc.vector

            def ts_op(tag, dst, src, scol, bcol):
                # dst = scol*src + bcol on pool/act/dve per placement
                if tag in act_ops:
                    nc.scalar.activation(dst, src, AF.Identity,
                                         bias=bcol if not isinstance(bcol, float)
                                         else bcol, scale=scol)
                else:
                    E(tag).tensor_scalar(dst, src, scol, bcol,
                                         OP.mult, OP.add)

            # ---------- op emitters (callable in any order) ----------
            # forms: f in {e0, e1, kl, y0, y1, y2}
            # e1 is 2-term: lead z1, aux z2. others: lead z0, aux z1 + z2.
            FORM = {
                "e0": (None, C_A1E0, C_BE0, C_A2E0, C_NA1E0, C_NBE0),
                "kl": (None, C_A1KL, C_BKL, C_A2KL, C_NA1KL, C_NBKL),
                "y0": (None, C_A1Y + 0, C_BY + 0, C_A2Y + 0, C_NA1Y, C_NBY),
                "y1": (None, C_A1Y + 1, C_BY + 1, C_A2Y + 1, C_NA1Y + 1,
                       C_NBY + 1),
                "y2": (None, C_A1Y + 2, C_BY + 2, C_A2Y + 2, C_NA1Y + 2,
                       C_NBY + 2),
            }

            def m1(f):      # tmp = a1*z1 + b   (aux mul with bias)
                _, a1, b, _, _, _ = FORM[f]
                ts_op(f + "m1", t(f + "_m1")[:, :], z1, col(a1), col(b))

            def a1(f):      # acc = z0 + tmp
                nc.vector.tensor_add(out=t(f + "_a")[:, :], in0=z0,
                                     in1=t(f + "_m1")[:, :])

            def ln1(f):     # acc = z0 + a1*z1 + b via custom op (1 DVE op)
                _, _, _, _, na1, nb = FORM[f]
                nc.vector.ln_bwd_dx(t(f + "_a")[:, :], z0, z1, col(na1),
                                    col(nb))

            def m2(f):      # tmp2 = a2*z2
                _, _, _, a2, _, _ = FORM[f]
                ts_op(f + "m2", t(f + "_m2")[:, :], z2, col(a2), 0.0)

            def a2(f):      # out = acc + tmp2
                E(f + "a2").tensor_add(out=t(f)[:, :], in0=t(f + "_a")[:, :],
                                       in1=t(f + "_m2")[:, :])

            def s2(f):      # out = acc + a2*z2 via stt (skip m2)
                _, _, _, a2c, _, _ = FORM[f]
                nc.vector.scalar_tensor_tensor(t(f)[:, :], z2, col(a2c),
                                               t(f + "_a")[:, :], OP.mult,
                                               OP.add)

            def e1m(_=None):   # e1 aux: tmp = a1*z2 + b
                ts_op("e1m1", t("e1_m1")[:, :], z2, col(C_A1E1),
                      col(C_BE1))

            def e1a(_=None):   # e1 = z1 + tmp
                nc.vector.tensor_add(out=t("e1")[:, :], in0=z1,
                                     in1=t("e1_m1")[:, :])

            def e1ln(_=None):
                nc.vector.ln_bwd_dx(t("e1")[:, :], z1, z2, col(C_NA1E1),
                                    col(C_NBE1))

            def sq(i):      # ACT squares: 0 <- e0, 1 <- e1, 2 <- z2
                src = {0: t("e0"), 1: t("e1")}.get(i)
                scol = {0: C_SQ0S, 1: C_SQ1S, 2: C_SQ2S}[i]
                dst = t(f"sq{i}")
                if i == 2:
                    nc.scalar.activation(dst[:, :], z2, AF.Square,
                                         bias=col(C_SQ2B), scale=col(scol))
                else:
                    nc.scalar.activation(dst[:, :], src[:, :], AF.Square,
                                         scale=col(scol))

            def d1(_=None):
                E("d1").tensor_add(out=t("d1")[:, :], in0=t("sq1")[:, :],
                                   in1=t("sq2")[:, :])

            def d2(_=None):
                E("d2").tensor_add(out=t("d2")[:, :], in0=t("d1")[:, :],
                                   in1=t("sq0")[:, :])

            def krbf(_=None):
                nc.scalar.activation(t("krbf")[:, :], t("d2")[:, :], AF.Exp,
                                     bias=col(C_BETA0), scale=col(C_NEGG))

            def p2(_=None):
                nc.scalar.activation(t("p2")[:, :], t("kl")[:, :], AF.Square,
                                     bias=col(C_P2B), scale=col(C_P2S))

            def tpoly(_=None):
                nc.vector.scalar_tensor_tensor(t("tpoly")[:, :],
                                               t("kl")[:, :], col(C_W1U0),
                                               t("p2")[:, :], OP.mult, op_w2)

            def tpm(_=None):
                ts_op("tpm", t("tp_m")[:, :], t("kl")[:, :], col(C_W1U0), 0.0)

            def tpa(_=None):
                if sw2_pos:
                    E("tpa").tensor_add(out=t("tpoly")[:, :],
                                        in0=t("tp_m")[:, :],
                                        in1=t("p2")[:, :])
                else:
                    E("tpa").tensor_sub(out=t("tpoly")[:, :],
                                        in0=t("tp_m")[:, :],
                                        in1=t("p2")[:, :])

            def t2(_=None):
                if sw0_pos:
                    E("t2").tensor_add(out=t("t2")[:, :],
                                       in0=t("tpoly")[:, :],
                                       in1=t("krbf")[:, :])
                else:
                    E("t2").tensor_sub(out=t("t2")[:, :],
                                       in0=t("tpoly")[:, :],
                                       in1=t("krbf")[:, :])

            def th(_=None):
                nc.scalar.activation(t("th")[:, :], t("t2")[:, :], AF.Tanh,
                                     scale=0.5)

            def g1(o):
                if o in g1_act:
                    nc.scalar.activation(t(f"g1{o}")[:, :], t("th")[:, :],
                                         AF.Identity, bias=col(C_G1B + o),
                                         scale=col(C_G1S + o))
                else:
                    E(f"g1{o}").tensor_scalar(t(f"g1{o}")[:, :],
                                              t("th")[:, :], col(C_G1S + o),
                                              col(C_G1B + o), OP.mult,
                                              OP.add)

            def v(o, nsplit=1):
                vt = t(f"v{o}")
                cw = FREE // nsplit
                for sdx in range(nsplit):
                    sl = (slice(None), slice(sdx * cw, (sdx + 1) * cw))
                    E(f"v{o}").tensor_mul(out=vt[sl], in0=t(f"y{o}")[sl],
                                          in1=t(f"g1{o}")[sl])
                    fin = vt
                    if not bo_zero:
                        fin = t(f"f{o}")
                        nc.vector.tensor_scalar(fin[sl], vt[sl], 1.0,
                                                col(C_BO + o), OP.mult,
                                                OP.add)
                    dmaeng(out_eng[o]).dma_start(out=outs[o][sl],
                                                 in_=fin[sl])

            # ---------- emission schedule ----------
            # Pool-assigned mul ops are emitted when their step comes up;
            # engine in-order sequencing follows emission order per engine.
            steps = {
                "e1m": e1m, "e1a": e1a, "e1ln": e1ln, "d1": d1, "d2": d2,
                "krbf": krbf, "p2": p2, "tpoly": tpoly, "tpm": tpm,
                "tpa": tpa, "t2": t2, "th": th,
            }
            for f in FORM:
                steps[f + "m1"] = (lambda ff: lambda _=None: m1(ff))(f)
                steps[f + "a1"] = (lambda ff: lambda _=None: a1(ff))(f)
                steps[f + "ln1"] = (lambda ff: lambda _=None: ln1(ff))(f)
                steps[f + "m2"] = (lambda ff: lambda _=None: m2(ff))(f)
                steps[f + "a2"] = (lambda ff: lambda _=None: a2(ff))(f)
                steps[f + "s2"] = (lambda ff: lambda _=None: s2(ff))(f)
            for o in range(3):
                steps[f"sq{o}"] = (lambda oo: lambda _=None: sq(oo))(o)
                steps[f"g1{o}"] = (lambda oo: lambda _=None: g1(oo))(o)
                steps[f"v{o}"] = (lambda oo: lambda _=None: v(oo))(o)
                steps[f"v{o}s"] = (lambda oo: lambda _=None: v(oo, 2))(o)

            if dve_order is None:
                dve_order = DEFAULT_ORDER
            for s in dve_order:
                steps[s]()
    nc.compile()
    return nc


# Default schedule: e1 first (z2 arrives early on pool queue), kl path
# early (feeds p2 before krbf), e0 path, squares interleave on ACT, y
# mac work fills DVE while ACT runs, tail g/v lanes. Pool-assigned ops
# appear in the order too (per-engine in-order follows emission order).
DEFAULT_ORDER = (
    "klm1", "e0m1", "y0m1",           # dve ts of z1 (start asap)
    "sq2", "e1m", "y0m2",             # act: z2 square + offloaded affines
    "e0m2", "klm2", "y1m1", "y2m1", "y1m2", "y2m2",   # pool ts queue
    "kla1", "e1a", "e0a1",
    "sq1",
    "kla2", "e0a2",
    "sq0",
    "y0a1", "d1", "d2",
    "p2", "krbf",
    "y1a1", "tpoly", "t2",
    "th",
    "y0a2", "y1a2", "y2a1", "y2a2",
    "g10", "g11", "g12",
    "v0", "v1", "v2",
)


def _get_nc(sw0_pos, sw2_pos, bo_zero, cfg=None):
    def freeze(v):
        if isinstance(v, dict):
            return tuple(sorted(v.items()))
        return v
    key = (sw0_pos, sw2_pos, bo_zero,
           tuple(sorted((k, freeze(v)) for k, v in (cfg or {}).items())))
    if key not in _NC_CACHE:
        _NC_CACHE[key] = _build_nc(sw0_pos, sw2_pos, bo_zero, cfg)
    return _NC_CACHE[key]


def _host_prep(inputs):
    d = {k: np.asarray(v, dtype=np.float64) for k, v in inputs.items()}
    z = np.asarray(inputs["z"], dtype=np.float32)
    B, C, H, W = z.shape
    Wz, bz = d["z_proj_w"], d["z_proj_b"]
    Wt, bt = d["text_proj_w"], d["text_proj_b"]
    Wo, bo = d["out_w"], d["out_b"]
    gamma = np.exp(d["log_gamma"])
    alpha, c, w = d["alpha"], d["c"], d["w"]
    sumw = w.sum() + 1e-8
    w0p, w1p, w2p = w[0] / sumw, w[1] / sumw, w[2] / sumw

    t = d["text_vec"] @ Wt.T + bt                      # [B, HID]
    G = Wz.T @ Wz
    L = np.linalg.cholesky(G)                          # may raise
    delta = bz[None, :] - t
    v = delta @ Wz                                     # [B, 3]
    cdist = (delta ** 2).sum(1)
    r = np.linalg.solve(L, v.T).T                      # [B, 3]
    rho = cdist - (r ** 2).sum(1)
    u = t @ Wz                                         # [B, 3]
    s = t @ bz                                         # [B]
    M = Wo @ Wz                                        # [3, 3]
    m = Wo @ bz                                        # [3]

    u0 = u[:, 0]
    piv = [L[0, 0], L[1, 1]] + [M[o, 0] for o in range(3)]
    if min(abs(np.asarray(piv))) < 1e-7 or np.any(np.abs(u0) < 1e-7):
        raise np.linalg.LinAlgError("degenerate pivot")

    if w0p == 0.0:
        beta0 = np.full(B, -1e30)
    else:
        beta0 = -gamma * rho + np.log(abs(w0p))
    sw2 = np.sqrt(abs(w2p))

    cb = np.zeros((B, NCONST), dtype=np.float64)
    cb[:, C_A1E0] = L[1, 0] / L[0, 0]
    cb[:, C_BE0] = r[:, 0] / L[0, 0]
    cb[:, C_A2E0] = L[2, 0] / L[0, 0]
    cb[:, C_SQ0S] = L[0, 0]
    cb[:, C_A1E1] = L[2, 1] / L[1, 1]
    cb[:, C_BE1] = r[:, 1] / L[1, 1]
    cb[:, C_SQ1S] = L[1, 1]
    cb[:, C_SQ2S] = L[2, 2]
    cb[:, C_SQ2B] = r[:, 2]
    cb[:, C_NEGG] = -gamma
    cb[:, C_BETA0] = beta0
    cb[:, C_A1KL] = u[:, 1] / u0
    cb[:, C_BKL] = s / u0
    cb[:, C_A2KL] = u[:, 2] / u0
    cb[:, C_P2S] = alpha * sw2 * u0
    cb[:, C_P2B] = c * sw2
    cb[:, C_W1U0] = w1p * u0
    for o in range(3):
        cb[:, C_A1Y + o] = M[o, 1] / M[o, 0]
        cb[:, C_BY + o] = m[o] / M[o, 0]
        cb[:, C_A2Y + o] = M[o, 2] / M[o, 0]
        cb[:, C_G1S + o] = 0.5 * M[o, 0]
        cb[:, C_G1B + o] = 1.5 * M[o, 0]
        cb[:, C_BO + o] = bo[o]
    # negated copies for ln_bwd (out = in0 - in1*s0 - s1)
    cb[:, C_NA1E0] = -cb[:, C_A1E0]
    cb[:, C_NBE0] = -cb[:, C_BE0]
    cb[:, C_NA1E1] = -cb[:, C_A1E1]
    cb[:, C_NBE1] = -cb[:, C_BE1]
    cb[:, C_NA1KL] = -cb[:, C_A1KL]
    cb[:, C_NBKL] = -cb[:, C_BKL]
    for o in range(3):
        cb[:, C_NA1Y + o] = -cb[:, C_A1Y + o]
        cb[:, C_NBY + o] = -cb[:, C_BY + o]
    cb = cb.astype(np.float32)

    z16 = z.astype(np.float16)
    in_maps = []
    for core in range(NCORES):
        cs = np.empty((P, NCONST), dtype=np.float32)
        z01a = np.empty((P, 2 * FREE), dtype=np.float16)
        z2a = np.empty((P, FREE), dtype=np.float16)
        for j in range(BPC):
            b = core * BPC + j
            pl = z16[b].reshape(3, ROWS, FREE)
            rs = slice(j * ROWS, (j + 1) * ROWS)
            z01a[rs, 0:FREE] = pl[1]
            z01a[rs, FREE:2 * FREE] = pl[0]
            z2a[rs, :] = pl[2]
            cs[rs, :] = cb[b]
        in_maps.append({"consts": cs, "z01": z01a, "z2": z2a})
    flags = (bool(w0p >= 0.0), bool(w2p >= 0.0),
             bool(np.all(bo == 0.0)))
    return in_maps, flags, (B, C, H, W)


def _numpy_fallback(inputs):
    d = {k: np.asarray(v, dtype=np.float64) for k, v in inputs.items()}
    z, Wz, bz = d["z"], d["z_proj_w"], d["z_proj_b"]
    t = d["text_vec"] @ d["text_proj_w"].T + d["text_proj_b"]
    zm = np.einsum("bchw,oc->bohw", z, Wz) + bz[None, :, None, None]
    gamma = np.exp(d["log_gamma"])
    diff = zm - t[:, :, None, None]
    dist = (diff * diff).sum(1)
    klin = np.einsum("bchw,bc->bhw", zm, t)
    krbf = np.exp(-gamma * dist)
    kpoly = (d["alpha"] * klin + d["c"]) ** 2
    w = d["w"]
    k = (w[0] * krbf + w[1] * klin + w[2] * kpoly) / (w.sum() + 1e-8)
    zf = zm * (1.0 + 1.0 / (1.0 + np.exp(-k[:, None])))
    out = np.einsum("bchw,oc->bohw", zf, d["out_w"]) + d["out_b"][None, :, None, None]
    return out.astype(np.float32)


def run(inputs, trace=False, cfg=None):
    if cfg is None:
        cfg = BEST_CFG
    try:
        in_maps, (sw0, sw2, boz), (B, C, H, W) = _host_prep(inputs)
    except np.linalg.LinAlgError:
        return _numpy_fallback(inputs), None
    nc = _get_nc(sw0, sw2, boz, cfg)
    res = bass_utils.run_bass_kernel_spmd(
        nc, in_maps, core_ids=list(range(NCORES)), trace=trace)
    out = np.empty((B, C, H, W), dtype=np.float32)
    for core in range(NCORES):
        r = res.results[core]
        for j in range(BPC):
            b = core * BPC + j
            rs = slice(j * ROWS, (j + 1) * ROWS)
            for o in range(3):
                out[b, o] = np.asarray(r[f"o{o}"][rs, :],
                                       dtype=np.float32).reshape(H, W)
    return out, res


def kernel(**inputs):
    out, _ = run(inputs, trace=False)
    return out


# revision 16
# speedup vs baseline: 1.6262x; 1.0344x over previous
"""Fused per-pixel kernel for nn_KernelFusion_19026705121450 on 8 trn2 cores.

Math: per pixel q = z[b,:,h,w] (3 ch), per batch t = Wt text + bt:
    z_map = Wz q + bz; dist = ||z_map - t||^2; kl = z_map . t
    k = (w0 e^{-g dist} + w1 kl + w2 (a kl + c)^2) / (sum w + 1e-8)
    out = Wo (z_map (1 + sigmoid(k))) + bo

All 64-dim reductions collapse (host, fp64) to 3-dim forms:
    dist = ||L^T q + r||^2 + rho   (L = chol(Wz^T Wz))
    kl   = u . q + s
    out_o = (M_o . q + m_o) g + bo_o,  M = Wo Wz, g = 1.5 + 0.5 tanh(k/2)

Device: one 1024-col pass over [128, 1024] fp16 tiles (partition =
batch*64 + rowblock, free = pixel). Forms are pivot-normalized on their
lead channel so biases ride tensor_scalar const slots; pivot scales
refold into ACT Square scales / per-o g1 consts. tanh (same ACT table
as exp/square) replaces sigmoid to avoid a table reload; a warmup ACT
op preloads the table before DMAs land. MACs decompose per cfg across
DVE (ts+tt / stt / ln_bwd custom op) and Pool (ts half).
"""

import sys

if "/opt/trn_rl_repo" not in sys.path:
    sys.path.insert(0, "/opt/trn_rl_repo")

import numpy as np

import concourse.bass as bass
import concourse.bacc as bacc
import concourse.mybir as mybir
from concourse.tile import TileContext
from concourse import bass_utils

F32 = mybir.dt.float32
F16 = mybir.dt.float16
AF = mybir.ActivationFunctionType
OP = mybir.AluOpType

NCORES = 8
BPC = 2          # batches per core
ROWS = 64        # partition rows per batch
P = 128
FREE = 1024

# const column indices (fp32 tensor)
# form f: z_lead + a1*z_a + a2*z_b + bias  (negated copies for ln mode)
C_A1E0, C_BE0, C_A2E0, C_SQ0S = 0, 1, 2, 3
C_A1E1, C_BE1, C_SQ1S = 4, 5, 6
C_SQ2S, C_SQ2B = 7, 8
C_NEGG, C_BETA0 = 9, 10
C_A1KL, C_BKL, C_A2KL = 11, 12, 13
C_P2S, C_P2B, C_W1U0 = 14, 15, 16
C_A1Y, C_BY, C_A2Y = 17, 20, 23       # +o
C_G1S, C_G1B = 26, 29                 # +o
C_BO = 32                             # +o
C_NA1E0, C_NBE0 = 35, 36              # negated (for ln_bwd mode)
C_NA1E1, C_NBE1 = 37, 38
C_NA1KL, C_NBKL = 39, 40
C_NA1Y, C_NBY = 41, 44                # +o
NCONST = 47

_NC_CACHE: dict = {}

# Best found schedule: dist path first (krbf fires early), y-form work
# fills DVE afterward, y0 muls ride ACT idle slots, outputs on the SP
# hardware-DGE queue. Measured 20661 ns in TimelineSim (8-core SPMD).
ORDER_BEST = (
    "klm1", "e0m1", "e1m",
    "e0m2", "klm2", "y1m1", "y2m1", "y1m2", "y2m2",
    "sq2",
    "e0a1", "kla1", "e1a",
    "sq1",
    "e0a2", "kla2",
    "sq0",
    "tpm", "d1", "d2",
    "p2", "krbf",
    "tpa", "t2",
    "th",
    "y0m1", "y0m2", "y0a1", "y1a1",
    "g11", "g12",
    "y0a2", "y1a2", "y2a1", "y2a2",
    "g10", "v0", "v1", "v2",
)

BEST_CFG: dict = {"in_eng": ("act", "sync", "sync", "sync"),
                  "zorder": "z2split", "act_ops": ("y0m1", "y0m2"),
                  "out_eng": ("sync", "sync", "sync"),
                  "dve_order": ORDER_BEST}


def _build_nc(sw0_pos: bool, sw2_pos: bool, bo_zero: bool, cfg: dict | None):
    """Emission order is hand-scheduled for the in-order engines.

    Step names (used by the `plan` cfg: list of (step, engine) pairs, where
    engine is 'dve'|'pool'|'act' for compute placement where it matters):
      mul ops ("<form>m1" = ts of z_aux w/ bias, "<form>m2" = ts of z2),
      adds ("<form>a1" lead+m1, "<form>a2" +m2), squares/exp/tanh on ACT
      fixed, d1/d2/t2 adds, tpoly stt, g1_o, v_o.
    """
    cfg = dict(cfg or {})
    warm = cfg.get("warm", True)
    # consts, z1, z0, z2 DMA queues
    in_eng = cfg.get("in_eng", ("act", "sync", "sync", "pool"))
    out_eng = cfg.get("out_eng", ("sync", "pool", "act"))
    # placement of the movable mul/aux ops
    pool_ops = set(cfg.get("pool_ops",
                           ("e0m2", "klm2", "y1m1", "y2m1", "y1m2",
                            "y2m2")))
    act_ops = set(cfg.get("act_ops", ("e1m1", "y0m2")))
    dve_order = cfg.get("dve_order", None)
    g1_act = set(cfg.get("g1_act", (1, 2)))     # g1 indices on ACT

    nc = bacc.Bacc("TRN2", target_bir_lowering=False)
    cons = nc.dram_tensor("consts", [P, NCONST], F32, kind="ExternalInput")
    z01 = nc.dram_tensor("z01", [P, 2 * FREE], F16, kind="ExternalInput")
    z2d = nc.dram_tensor("z2", [P, FREE], F16, kind="ExternalInput")
    outs = [nc.dram_tensor(f"o{o}", [P, FREE], F16, kind="ExternalOutput")
            for o in range(3)]

    op_w2 = OP.add if sw2_pos else OP.subtract

    def dmaeng(which):
        return {"sync": nc.sync, "pool": nc.gpsimd, "act": nc.scalar,
                "dve": nc.vector}[which]

    with TileContext(nc) as tc:
        with tc.tile_pool(name="cpool", bufs=1) as cpool, \
             tc.tile_pool(name="work", bufs=1) as pool:
            ct = cpool.tile([P, NCONST], F32, name="ct")
            zt = cpool.tile([P, 2 * FREE], F16, name="zt")
            z2t = cpool.tile([P, FREE], F16, name="z2t")
            wt = cpool.tile([P, 1], F32, name="wt")

            if warm:
                nc.vector.memset(wt[:, :], 0.0)
                nc.scalar.activation(wt[:, :], wt[:, :], AF.Square)

            # packed z01 = [z1 | z0]
            zorder = cfg.get("zorder", "z2first")
            dmaeng(in_eng[0]).dma_start(out=ct[:, :], in_=cons[:, :])
            if zorder == "z2first":
                dmaeng(in_eng[3]).dma_start(out=z2t[:, :], in_=z2d[:, :])
                dmaeng(in_eng[1]).dma_start(out=zt[:, :], in_=z01[:, :])
            elif zorder == "z01first":
                dmaeng(in_eng[1]).dma_start(out=zt[:, :], in_=z01[:, :])
                dmaeng(in_eng[3]).dma_start(out=z2t[:, :], in_=z2d[:, :])
            elif zorder == "split":
                dmaeng(in_eng[1]).dma_start(out=zt[:, 0:FREE],
                                            in_=z01[:, 0:FREE])
                dmaeng(in_eng[3]).dma_start(out=z2t[:, :], in_=z2d[:, :])
                dmaeng(in_eng[2]).dma_start(out=zt[:, FREE:2 * FREE],
                                            in_=z01[:, FREE:2 * FREE])
            elif zorder == "z2split":  # z2, z1, z0 (all split)
                dmaeng(in_eng[3]).dma_start(out=z2t[:, :], in_=z2d[:, :])
                dmaeng(in_eng[1]).dma_start(out=zt[:, 0:FREE],
                                            in_=z01[:, 0:FREE])
                dmaeng(in_eng[2]).dma_start(out=zt[:, FREE:2 * FREE],
                                            in_=z01[:, FREE:2 * FREE])
            else:  # z1first: z1, z2, z0
                dmaeng(in_eng[1]).dma_start(out=zt[:, 0:FREE],
                                            in_=z01[:, 0:FREE])
                dmaeng(in_eng[3]).dma_start(out=z2t[:, :], in_=z2d[:, :])
                dmaeng(in_eng[2]).dma_start(out=zt[:, FREE:2 * FREE],
                                            in_=z01[:, FREE:2 * FREE])
            z1 = zt[:, 0:FREE]
            z0 = zt[:, FREE:2 * FREE]
            z2 = z2t[:, :]

            def col(j):
                return ct[:, j:j + 1]

            tiles = {}

            def t(tag, w=FREE):
                if tag not in tiles:
                    tiles[tag] = pool.tile([P, w], F16, tag=tag, name=tag)
                return tiles[tag]

            def E(tag):
                return nc.gpsimd if tag in pool_ops else nc.vector

            def ts_op(tag, dst, src, scol, bcol):
                # dst = scol*src + bcol on pool/act/dve per placement
                if tag in act_ops:
                    nc.scalar.activation(dst, src, AF.Identity,
                                         bias=bcol if not isinstance(bcol, float)
                                         else bcol, scale=scol)
                else:
                    E(tag).tensor_scalar(dst, src, scol, bcol,
                                         OP.mult, OP.add)

            # ---------- op emitters (callable in any order) ----------
            # forms: f in {e0, e1, kl, y0, y1, y2}
            # e1 is 2-term: lead z1, aux z2. others: lead z0, aux z1 + z2.
            FORM = {
                "e0": (None, C_A1E0, C_BE0, C_A2E0, C_NA1E0, C_NBE0),
                "kl": (None, C_A1KL, C_BKL, C_A2KL, C_NA1KL, C_NBKL),
                "y0": (None, C_A1Y + 0, C_BY + 0, C_A2Y + 0, C_NA1Y, C_NBY),
                "y1": (None, C_A1Y + 1, C_BY + 1, C_A2Y + 1, C_NA1Y + 1,
                       C_NBY + 1),
                "y2": (None, C_A1Y + 2, C_BY + 2, C_A2Y + 2, C_NA1Y + 2,
                       C_NBY + 2),
            }

            def m1(f):      # tmp = a1*z1 + b   (aux mul with bias)
                _, a1, b, _, _, _ = FORM[f]
                ts_op(f + "m1", t(f + "_m1")[:, :], z1, col(a1), col(b))

            def a1(f):      # acc = z0 + tmp
                nc.vector.tensor_add(out=t(f + "_a")[:, :], in0=z0,
                                     in1=t(f + "_m1")[:, :])

            def ln1(f):     # acc = z0 + a1*z1 + b via custom op (1 DVE op)
                _, _, _, _, na1, nb = FORM[f]
                nc.vector.ln_bwd_dx(t(f + "_a")[:, :], z0, z1, col(na1),
                                    col(nb))

            def m2(f):      # tmp2 = a2*z2
                _, _, _, a2, _, _ = FORM[f]
                ts_op(f + "m2", t(f + "_m2")[:, :], z2, col(a2), 0.0)

            def a2(f):      # out = acc + tmp2
                E(f + "a2").tensor_add(out=t(f)[:, :], in0=t(f + "_a")[:, :],
                                       in1=t(f + "_m2")[:, :])

            def s2(f):      # out = acc + a2*z2 via stt (skip m2)
                _, _, _, a2c, _, _ = FORM[f]
                nc.vector.scalar_tensor_tensor(t(f)[:, :], z2, col(a2c),
                                               t(f + "_a")[:, :], OP.mult,
                                               OP.add)

            def e1m(_=None):   # e1 aux: tmp = a1*z2 + b
                ts_op("e1m1", t("e1_m1")[:, :], z2, col(C_A1E1),
                      col(C_BE1))

            def e1a(_=None):   # e1 = z1 + tmp
                nc.vector.tensor_add(out=t("e1")[:, :], in0=z1,
                                     in1=t("e1_m1")[:, :])

            def e1ln(_=None):
                nc.vector.ln_bwd_dx(t("e1")[:, :], z1, z2, col(C_NA1E1),
                                    col(C_NBE1))

            def sq(i):      # ACT squares: 0 <- e0, 1 <- e1, 2 <- z2
                src = {0: t("e0"), 1: t("e1")}.get(i)
                scol = {0: C_SQ0S, 1: C_SQ1S, 2: C_SQ2S}[i]
                dst = t(f"sq{i}")
                if i == 2:
                    nc.scalar.activation(dst[:, :], z2, AF.Square,
                                         bias=col(C_SQ2B), scale=col(scol))
                else:
                    nc.scalar.activation(dst[:, :], src[:, :], AF.Square,
                                         scale=col(scol))

            def d1(_=None):
                E("d1").tensor_add(out=t("d1")[:, :], in0=t("sq1")[:, :],
                                   in1=t("sq2")[:, :])

            def d2(_=None):
                E("d2").tensor_add(out=t("d2")[:, :], in0=t("d1")[:, :],
                                   in1=t("sq0")[:, :])

            def krbf(_=None):
                nc.scalar.activation(t("krbf")[:, :], t("d2")[:, :], AF.Exp,
                                     bias=col(C_BETA0), scale=col(C_NEGG))

            def p2(_=None):
                nc.scalar.activation(t("p2")[:, :], t("kl")[:, :], AF.Square,
                                     bias=col(C_P2B), scale=col(C_P2S))

            def tpoly(_=None):
                nc.vector.scalar_tensor_tensor(t("tpoly")[:, :],
                                               t("kl")[:, :], col(C_W1U0),
                                               t("p2")[:, :], OP.mult, op_w2)

            def tpm(_=None):
                ts_op("tpm", t("tp_m")[:, :], t("kl")[:, :], col(C_W1U0), 0.0)

            def tpa(_=None):
                if sw2_pos:
                    E("tpa").tensor_add(out=t("tpoly")[:, :],
                                        in0=t("tp_m")[:, :],
                                        in1=t("p2")[:, :])
                else:
                    E("tpa").tensor_sub(out=t("tpoly")[:, :],
                                        in0=t("tp_m")[:, :],
                                        in1=t("p2")[:, :])

            def t2(_=None):
                if sw0_pos:
                    E("t2").tensor_add(out=t("t2")[:, :],
                                       in0=t("tpoly")[:, :],
                                       in1=t("krbf")[:, :])
                else:
                    E("t2").tensor_sub(out=t("t2")[:, :],
                                       in0=t("tpoly")[:, :],
                                       in1=t("krbf")[:, :])

            def th(_=None):
                nc.scalar.activation(t("th")[:, :], t("t2")[:, :], AF.Tanh,
                                     scale=0.5)

            def g1(o):
                if o in g1_act:
                    nc.scalar.activation(t(f"g1{o}")[:, :], t("th")[:, :],
                                         AF.Identity, bias=col(C_G1B + o),
                                         scale=col(C_G1S + o))
                else:
                    E(f"g1{o}").tensor_scalar(t(f"g1{o}")[:, :],
                                              t("th")[:, :], col(C_G1S + o),
                                              col(C_G1B + o), OP.mult,
                                              OP.add)

            def v(o, nsplit=1):
                vt = t(f"v{o}")
                cw = FREE // nsplit
                for sdx in range(nsplit):
                    sl = (slice(None), slice(sdx * cw, (sdx + 1) * cw))
                    E(f"v{o}").tensor_mul(out=vt[sl], in0=t(f"y{o}")[sl],
                                          in1=t(f"g1{o}")[sl])
                    fin = vt
                    if not bo_zero:
                        fin = t(f"f{o}")
                        nc.vector.tensor_scalar(fin[sl], vt[sl], 1.0,
                                                col(C_BO + o), OP.mult,
                                                OP.add)
                    dmaeng(out_eng[o]).dma_start(out=outs[o][sl],
                                                 in_=fin[sl])

            # ---------- emission schedule ----------
            # Pool-assigned mul ops are emitted when their step comes up;
            # engine in-order sequencing follows emission order per engine.
            steps = {
                "e1m": e1m, "e1a": e1a, "e1ln": e1ln, "d1": d1, "d2": d2,
                "krbf": krbf, "p2": p2, "tpoly": tpoly, "tpm": tpm,
                "tpa": tpa, "t2": t2, "th": th,
            }
            for f in FORM:
                steps[f + "m1"] = (lambda ff: lambda _=None: m1(ff))(f)
                steps[f + "a1"] = (lambda ff: lambda _=None: a1(ff))(f)
                steps[f + "ln1"] = (lambda ff: lambda _=None: ln1(ff))(f)
                steps[f + "m2"] = (lambda ff: lambda _=None: m2(ff))(f)
                steps[f + "a2"] = (lambda ff: lambda _=None: a2(ff))(f)
                steps[f + "s2"] = (lambda ff: lambda _=None: s2(ff))(f)
            for o in range(3):
                steps[f"sq{o}"] = (lambda oo: lambda _=None: sq(oo))(o)
                steps[f"g1{o}"] = (lambda oo: lambda _=None: g1(oo))(o)
                steps[f"v{o}"] = (lambda oo: lambda _=None: v(oo))(o)
                steps[f"v{o}s"] = (lambda oo: lambda _=None: v(oo, 2))(o)

            if dve_order is None:
                dve_order = DEFAULT_ORDER
            for s in dve_order:
                steps[s]()
    nc.compile()
    return nc


# Default schedule: e1 first (z2 arrives early on pool queue), kl path
# early (feeds p2 before krbf), e0 path, squares interleave on ACT, y
# mac work fills DVE while ACT runs, tail g/v lanes. Pool-assigned ops
# appear in the order too (per-engine in-order follows emission order).
DEFAULT_ORDER = (
    "klm1", "e0m1", "y0m1",           # dve ts of z1 (start asap)
    "sq2", "e1m", "y0m2",             # act: z2 square + offloaded affines
    "e0m2", "klm2", "y1m1", "y2m1", "y1m2", "y2m2",   # pool ts queue
    "kla1", "e1a", "e0a1",
    "sq1",
    "kla2", "e0a2",
    "sq0",
    "y0a1", "d1", "d2",
    "p2", "krbf",
    "y1a1", "tpoly", "t2",
    "th",
    "y0a2", "y1a2", "y2a1", "y2a2",
    "g10", "g11", "g12",
    "v0", "v1", "v2",
)


def _get_nc(sw0_pos, sw2_pos, bo_zero, cfg=None):
    def freeze(v):
        if isinstance(v, dict):
            return tuple(sorted(v.items()))
        return v
    key = (sw0_pos, sw2_pos, bo_zero,
           tuple(sorted((k, freeze(v)) for k, v in (cfg or {}).items())))
    if key not in _NC_CACHE:
        _NC_CACHE[key] = _build_nc(sw0_pos, sw2_pos, bo_zero, cfg)
    return _NC_CACHE[key]


def _host_prep(inputs):
    d = {k: np.asarray(v, dtype=np.float64) for k, v in inputs.items()}
    z = np.asarray(inputs["z"], dtype=np.float32)
    B, C, H, W = z.shape
    Wz, bz = d["z_proj_w"], d["z_proj_b"]
    Wt, bt = d["text_proj_w"], d["text_proj_b"]
    Wo, bo = d["out_w"], d["out_b"]
    gamma = np.exp(d["log_gamma"])
    alpha, c, w = d["alpha"], d["c"], d["w"]
    sumw = w.sum() + 1e-8
    w0p, w1p, w2p = w[0] / sumw, w[1] / sumw, w[2] / sumw

    t = d["text_vec"] @ Wt.T + bt                      # [B, HID]
    G = Wz.T @ Wz
    L = np.linalg.cholesky(G)                          # may raise
    delta = bz[None, :] - t
    v = delta @ Wz                                     # [B, 3]
    cdist = (delta ** 2).sum(1)
    r = np.linalg.solve(L, v.T).T                      # [B, 3]
    rho = cdist - (r ** 2).sum(1)
    u = t @ Wz                                         # [B, 3]
    s = t @ bz                                         # [B]
    M = Wo @ Wz                                        # [3, 3]
    m = Wo @ bz                                        # [3]

    u0 = u[:, 0]
    piv = [L[0, 0], L[1, 1]] + [M[o, 0] for o in range(3)]
    if min(abs(np.asarray(piv))) < 1e-7 or np.any(np.abs(u0) < 1e-7):
        raise np.linalg.LinAlgError("degenerate pivot")

    if w0p == 0.0:
        beta0 = np.full(B, -1e30)
    else:
        beta0 = -gamma * rho + np.log(abs(w0p))
    sw2 = np.sqrt(abs(w2p))

    cb = np.zeros((B, NCONST), dtype=np.float64)
    cb[:, C_A1E0] = L[1, 0] / L[0, 0]
    cb[:, C_BE0] = r[:, 0] / L[0, 0]
    cb[:, C_A2E0] = L[2, 0] / L[0, 0]
    cb[:, C_SQ0S] = L[0, 0]
    cb[:, C_A1E1] = L[2, 1] / L[1, 1]
    cb[:, C_BE1] = r[:, 1] / L[1, 1]
    cb[:, C_SQ1S] = L[1, 1]
    cb[:, C_SQ2S] = L[2, 2]
    cb[:, C_SQ2B] = r[:, 2]
    cb[:, C_NEGG] = -gamma
    cb[:, C_BETA0] = beta0
    cb[:, C_A1KL] = u[:, 1] / u0
    cb[:, C_BKL] = s / u0
    cb[:, C_A2KL] = u[:, 2] / u0
    cb[:, C_P2S] = alpha * sw2 * u0
    cb[:, C_P2B] = c * sw2
    cb[:, C_W1U0] = w1p * u0
    for o in range(3):
        cb[:, C_A1Y + o] = M[o, 1] / M[o, 0]
        cb[:, C_BY + o] = m[o] / M[o, 0]
        cb[:, C_A2Y + o] = M[o, 2] / M[o, 0]
        cb[:, C_G1S + o] = 0.5 * M[o, 0]
        cb[:, C_G1B + o] = 1.5 * M[o, 0]
        cb[:, C_BO + o] = bo[o]
    # negated copies for ln_bwd (out = in0 - in1*s0 - s1)
    cb[:, C_NA1E0] = -cb[:, C_A1E0]
    cb[:, C_NBE0] = -cb[:, C_BE0]
    cb[:, C_NA1E1] = -cb[:, C_A1E1]
    cb[:, C_NBE1] = -cb[:, C_BE1]
    cb[:, C_NA1KL] = -cb[:, C_A1KL]
    cb[:, C_NBKL] = -cb[:, C_BKL]
    for o in range(3):
        cb[:, C_NA1Y + o] = -cb[:, C_A1Y + o]
        cb[:, C_NBY + o] = -cb[:, C_BY + o]
    cb = cb.astype(np.float32)

    z16 = z.astype(np.float16)
    in_maps = []
    for core in range(NCORES):
        cs = np.empty((P, NCONST), dtype=np.float32)
        z01a = np.empty((P, 2 * FREE), dtype=np.float16)
        z2a = np.empty((P, FREE), dtype=np.float16)
        for j in range(BPC):
            b = core * BPC + j
            pl = z16[b].reshape(3, ROWS, FREE)
            rs = slice(j * ROWS, (j + 1) * ROWS)
            z01a[rs, 0:FREE] = pl[1]
            z01a[rs, FREE:2 * FREE] = pl[0]
            z2a[rs, :] = pl[2]
            cs[rs, :] = cb[b]
        in_maps.append({"consts": cs, "z01": z01a, "z2": z2a})
    flags = (bool(w0p >= 0.0), bool(w2p >= 0.0),
             bool(np.all(bo == 0.0)))
    return in_maps, flags, (B, C, H, W)


def _numpy_fallback(inputs):
    d = {k: np.asarray(v, dtype=np.float64) for k, v in inputs.items()}
    z, Wz, bz = d["z"], d["z_proj_w"], d["z_proj_b"]
    t = d["text_vec"] @ d["text_proj_w"].T + d["text_proj_b"]
    zm = np.einsum("bchw,oc->bohw", z, Wz) + bz[None, :, None, None]
    gamma = np.exp(d["log_gamma"])
    diff = zm - t[:, :, None, None]
    dist = (diff * diff).sum(1)
    klin = np.einsum("bchw,bc->bhw", zm, t)
    krbf = np.exp(-gamma * dist)
    kpoly = (d["alpha"] * klin + d["c"]) ** 2
    w = d["w"]
    k = (w[0] * krbf + w[1] * klin + w[2] * kpoly) / (w.sum() + 1e-8)
    zf = zm * (1.0 + 1.0 / (1.0 + np.exp(-k[:, None])))
    out = np.einsum("bchw,oc->bohw", zf, d["out_w"]) + d["out_b"][None, :, None, None]
    return out.astype(np.float32)


def run(inputs, trace=False, cfg=None):
    if cfg is None:
        cfg = BEST_CFG
    try:
        in_maps, (sw0, sw2, boz), (B, C, H, W) = _host_prep(inputs)
    except np.linalg.LinAlgError:
        return _numpy_fallback(inputs), None
    nc = _get_nc(sw0, sw2, boz, cfg)
    res = bass_utils.run_bass_kernel_spmd(
        nc, in_maps, core_ids=list(range(NCORES)), trace=trace)
    out = np.empty((B, C, H, W), dtype=np.float32)
    for core in range(NCORES):
        r = res.results[core]
        for j in range(BPC):
            b = core * BPC + j
            rs = slice(j * ROWS, (j + 1) * ROWS)
            for o in range(3):
                out[b, o] = np.asarray(r[f"o{o}"][rs, :],
                                       dtype=np.float32).reshape(H, W)
    return out, res


def kernel(**inputs):
    out, _ = run(inputs, trace=False)
    return out
